# revision 12
# baseline (speedup 1.0000x reference)
"""Trainium2 Bass kernel for a 4-layer causal decoder (V=32000, E=1024, H=16,
S=1024, batch 4) on 8 NeuronCores.

Sharding: core c -> (batch b=c//2, sequence-half h=c%2). Each core owns 512
tokens through the transformer layers; K/V are computed for the full sequence
locally from an AllGather'd hidden state (one 2-rank AllGather per layer
boundary, inside each batch pair). The vocab projection is column-split: after
a final AllGather each core computes logits for all 1024 tokens of its batch
x a 16000-wide vocab slice.

Layout: activations feature-major [E(partitions), tokens(free)]. Attention
scores are computed transposed S^T[k_tok, q_tok] so softmax probabilities come
out in the layout the A@V matmul consumes directly (no PE transposes).
Softmax denominators come from an appended ones-column on V (M=65 matmuls).
LayerNorm mean/var come from ones-vector matmuls; per-token rows are broadcast
across partitions with K=1 ones-matmuls.

Precision: fp32 storage with float32r matmuls (full PE rate at N>=256,
~1e-4 relative rounding). FFN2 (W2 @ relu) runs in bf16 to halve its SBUF/DMA
footprint.
"""

import numpy as np
from contextlib import ExitStack

import ml_dtypes

from concourse import bacc, bass, mybir, tile
from concourse.bass_utils import run_bass_kernel_spmd

F32 = mybir.dt.float32
F32R = mybir.dt.float32r
BF16 = mybir.dt.bfloat16
AF = mybir.ActivationFunctionType
OP = mybir.AluOpType

P = 128
V, E, L, H, S = 32000, 1024, 4, 16, 1024
D = E // H          # 64
FF = 4 * E          # 4096
NB = 4              # batch
NCORES = 8
T = S // 2          # 512 local tokens
VS = V // 2         # 16000 vocab slice per core
VC = 500            # vocab chunk (32 chunks)
NVC = VS // VC      # 32
ET = E // P         # 8 e-tiles
ST = S // P         # 8 token tiles
FT = FF // P        # 32 ffn tiles
NG = 4              # head groups
HPG = H // NG       # 4 heads per group
EPS = 1e-5
MASKVAL = -30000.0

# bias_pack vector order
BQ, BK, BO, B2, L1G, L1B, L2G, L2B = range(8)

_CACHE = {}
LAST_RESULTS = None


def _build():
    nc = bacc.Bacc(trn_type="TRN2", target_bir_lowering=False, debug=False,
                   num_devices=NCORES)

    # ---- DRAM I/O ----
    h0loc = nc.dram_tensor("h0loc", [E, T], F32R, kind="ExternalInput")
    h0full = nc.dram_tensor("h0full", [E, S], F32R, kind="ExternalInput")
    maskT = nc.dram_tensor("maskT", [S, T], BF16, kind="ExternalInput")
    wq = nc.dram_tensor("wq", [L, ET, P, ET, P], F32R, kind="ExternalInput")
    wk = nc.dram_tensor("wk", [L, ET, P, ET, P], F32R, kind="ExternalInput")
    wv = nc.dram_tensor("wv", [L, NG, ET, P, HPG * D], F32R,
                        kind="ExternalInput")
    wo = nc.dram_tensor("wo", [L, ET, P, ET, P], F32R, kind="ExternalInput")
    w1 = nc.dram_tensor("w1", [L, FT, P, ET, P], F32R, kind="ExternalInput")
    w2 = nc.dram_tensor("w2", [L, ET, 2, P, (FT // 2) * P], BF16,
                        kind="ExternalInput")
    bias8 = nc.dram_tensor("bias8", [L, P, 8, ET], F32, kind="ExternalInput")
    b1v = nc.dram_tensor("b1v", [L, P, FT], F32, kind="ExternalInput")
    bvv = nc.dram_tensor("bvv", [L, E], F32, kind="ExternalInput")
    woutS = nc.dram_tensor("woutS", [NVC, ET, P, VC], F32R,
                           kind="ExternalInput")
    boutS = nc.dram_tensor("boutS", [VS], F32, kind="ExternalInput")
    onesc = nc.dram_tensor("onesc", [P, P], F32R, kind="ExternalInput")
    logits = nc.dram_tensor("logits", [S, VS], F32, kind="ExternalOutput")

    def bcast_rows(ap1d, nparts):
        # [N] DRAM AP -> [nparts, N] with zero partition stride
        return bass.AP(tensor=ap1d.tensor, offset=ap1d.offset,
                       ap=[[0, nparts]] + [list(d) for d in ap1d.ap])

    EH = E // 2
    cc_in = [[nc.dram_tensor(f"cc_in{i}_{h}", [EH, T], F32R) for h in range(2)]
             for i in range(L)]
    cc_out = [[nc.dram_tensor(f"cc_out{i}_{h}", [2, EH, T], F32R)
               for h in range(2)] for i in range(L)]
    groups = [[0, 1], [2, 3], [4, 5], [6, 7]]

    with tile.TileContext(nc) as tc, ExitStack() as ctx:
        def pool(name, bufs, space="SBUF"):
            return ctx.enter_context(
                tc.tile_pool(name=name, bufs=bufs, space=space))

        ph_in = pool("ph_in", 8)      # [128,1024] f32r  full-seq hidden
        ph_loc = pool("ph_loc", 8)    # [128,512]  f32r  local hidden
        pres = pool("pres", 8)        # [128,512]  f32r  res1/hln1 (+vocab bias bc)
        po = pool("po", 8)            # [128,512]        o_attn / res2 / vocab out
        pq = pool("pq", 2)            # [128,512]  f32r  Q tiles
        pk = pool("pk", 2)            # [128,1024] f32r  K tiles
        pv = pool("pv", 8)            # [128,260]  f32r  V_aug tiles
        pp = pool("pp", 2)            # [128,512]        exp probs / squares
        pf = pool("pf", 32)           # [128,512]  bf16  FFN relu acts
        pmask = pool("pmask", 8)      # [128,512]  bf16
        pw = pool("pw", 2)            # [128,8,128] f32r weight slabs
        pwv = pool("pwv", 8)          # [128,256]  f32r  Wv rhs tiles
        pw2 = pool("pw2", 2)          # [128,16,128] bf16 W2 slabs
        pwo = pool("pwo", 8)          # [128,500]  f32r  Wout rhs tiles
        pb = pool("pb", 2)            # small bias tiles
        prow = pool("prow", 4)        # [1,512] rows
        pone = pool("pone", 1)
        prbc = pool("prbc", 1)        # [64,512]

        psA = pool("psA", 4, space="PSUM")   # [128,512] f32
        psO = pool("psO", 2, space="PSUM")   # [65,512] f32
        psS = pool("psS", 2, space="PSUM")   # [1,512] f32

        # ---- constants ----
        ones_sb = pone.tile([P, P], F32R, tag="ones", bufs=1, name="ones_sb")
        nc.sync.dma_start(out=ones_sb, in_=onesc.ap())
        eps_t = pone.tile([1, 1], F32, tag="eps", bufs=1, name="eps_t")
        nc.vector.memset(eps_t, EPS)

        masks = []
        for kt in range(ST):
            mt = pmask.tile([P, T], BF16, tag="mask", bufs=8, name=f"mask{kt}")
            nc.sync.dma_start(out=mt, in_=maskT.ap()[kt * P:(kt + 1) * P, :])
            masks.append(mt)

        # ---- initial hidden ----
        h_loc = []
        h_in = []
        for e in range(ET):
            hl = ph_loc.tile([P, T], F32R, tag="hloc", bufs=8, name=f"hl0_{e}")
            nc.sync.dma_start(out=hl, in_=h0loc.ap()[e * P:(e + 1) * P, :])
            h_loc.append(hl)
            hf = ph_in.tile([P, S], F32R, tag="hin", bufs=8, name=f"hf0_{e}")
            nc.sync.dma_start(out=hf, in_=h0full.ap()[e * P:(e + 1) * P, :])
            h_in.append(hf)

        def layernorm(src, bias_t, gcol, bcol, out_tiles):
            """src: list of 8 [128,T] f32r tiles; normalizes over features.
            out_tiles: None -> in-place into src; else writes there."""
            ps_sum = psS.tile([1, T], F32, tag="psS", bufs=2, name="ps_sum")
            ps_sq = psS.tile([1, T], F32, tag="psS", bufs=2, name="ps_sq")
            for e in range(ET):
                sq = pp.tile([P, T], F32R, tag="pp", bufs=2, name=f"sq{e}")
                nc.vector.tensor_mul(sq, src[e], src[e])
                nc.tensor.matmul(ps_sum, ones_sb[:, 0:1], src[e],
                                 start=(e == 0), stop=(e == ET - 1))
                nc.tensor.matmul(ps_sq, ones_sb[:, 0:1], sq,
                                 start=(e == 0), stop=(e == ET - 1))
            mu = prow.tile([1, T], F32, tag="row", bufs=4, name="mu")
            nc.scalar.mul(mu, ps_sum, 1.0 / E)
            m2 = prow.tile([1, T], F32, tag="row", bufs=4, name="m2")
            nc.scalar.mul(m2, ps_sq, 1.0 / E)
            var = prow.tile([1, T], F32, tag="row", bufs=4, name="var")
            nc.vector.tensor_mul(var, mu, mu)
            nc.vector.tensor_sub(var, m2, var)
            sd = prow.tile([1, T], F32, tag="row", bufs=4, name="sd")
            nc.scalar.activation(sd, var, AF.Sqrt, bias=eps_t)
            rstd = prow.tile([1, T], F32R, tag="row", bufs=4, name="rstd")
            with nc.allow_low_precision(reason="f32r rows feed broadcast matmuls"):
                nc.vector.reciprocal(rstd, sd)
            crow = prow.tile([1, T], F32R, tag="row", bufs=4, name="crow")
            nc.vector.tensor_mul(crow, mu, rstd)
            ps_a = psA.tile([P, T], F32, tag="psA", bufs=4, name="ps_a")
            nc.tensor.matmul(ps_a, ones_sb[0:1, :], rstd, start=True, stop=True)
            ps_c = psA.tile([P, T], F32, tag="psA", bufs=4, name="ps_c")
            nc.tensor.matmul(ps_c, ones_sb[0:1, :], crow, start=True, stop=True)
            for e in range(ET):
                dst = src[e] if out_tiles is None else out_tiles[e]
                nc.vector.tensor_mul(dst, src[e], ps_a)
                nc.vector.tensor_sub(dst, dst, ps_c)
                nc.vector.tensor_scalar(
                    out=dst, in0=dst,
                    scalar1=bias_t[:, gcol, e:e + 1],
                    scalar2=bias_t[:, bcol, e:e + 1],
                    op0=OP.mult, op1=OP.add)

        # =================== layers ===================
        for i in range(L):
            b8 = pb.tile([P, 8, ET], F32, tag="b8", bufs=2, name=f"b8_{i}")
            nc.sync.dma_start(out=b8, in_=bias8.ap()[i])
            b1t = pb.tile([P, FT], F32, tag="b1t", bufs=2, name=f"b1t_{i}")
            nc.sync.dma_start(out=b1t, in_=b1v.ap()[i])
            bv_bc = []
            for chk in range(2):
                t_ = pb.tile([P, T], F32, tag="bvbc", bufs=2, name=f"bv{i}_{chk}")
                nc.sync.dma_start(
                    out=t_,
                    in_=bcast_rows(bvv.ap()[i, chk * T:(chk + 1) * T], P))
                bv_bc.append(t_)

            o_attn = [po.tile([P, T], F32R, tag="po", bufs=8, name=f"oat{i}_{e}")
                      for e in range(ET)]

            for g in range(NG):
                e0 = 2 * g  # first feature tile of this group
                # ---- Q for group (local tokens) ----
                q_tiles = []
                for fo in range(2):
                    fi = e0 + fo
                    wsl = pw.tile([P, ET, P], F32R, tag="pw", bufs=2,
                                  name=f"wq{i}_{fi}")
                    nc.sync.dma_start(out=wsl, in_=wq.ap()[i, fi])
                    qt = pq.tile([P, T], F32R, tag="pq", bufs=2,
                                 name=f"Q{i}_{fi}")
                    ps = psA.tile([P, T], F32, tag="psA", bufs=4, name="ps_q")
                    for e in range(ET):
                        nc.tensor.matmul(ps, wsl[:, e, :], h_loc[e],
                                         start=(e == 0), stop=(e == ET - 1))
                    nc.vector.tensor_scalar_add(qt, ps, b8[:, BQ, fi:fi + 1])
                    q_tiles.append(qt)
                # ---- K for group (full sequence) ----
                k_tiles = []
                for fo in range(2):
                    fi = e0 + fo
                    wsl = pw.tile([P, ET, P], F32R, tag="pw", bufs=2,
                                  name=f"wk{i}_{fi}")
                    nc.sync.dma_start(out=wsl, in_=wk.ap()[i, fi])
                    kt_sb = pk.tile([P, S], F32R, tag="pk", bufs=2,
                                    name=f"K{i}_{fi}")
                    for chk in range(2):
                        ps = psA.tile([P, T], F32, tag="psA", bufs=4,
                                      name="ps_k")
                        for e in range(ET):
                            nc.tensor.matmul(
                                ps, wsl[:, e, :],
                                h_in[e][:, chk * T:(chk + 1) * T],
                                start=(e == 0), stop=(e == ET - 1))
                        nc.vector.tensor_scalar_add(
                            kt_sb[:, chk * T:(chk + 1) * T], ps,
                            b8[:, BK, fi:fi + 1])
                    k_tiles.append(kt_sb)
                # ---- V for group (token-major, +ones col) ----
                wv_t = []
                for e in range(ET):
                    wt = pwv.tile([P, HPG * D], F32R, tag="pwv", bufs=8,
                                  name=f"wv{i}_{g}_{e}")
                    nc.sync.dma_start(out=wt, in_=wv.ap()[i, g, e])
                    wv_t.append(wt)
                v_tiles = []
                for tt in range(ST):
                    vt = pv.tile([P, HPG, D + 1], F32R, tag="pv", bufs=8,
                                 name=f"V{i}_{g}_{tt}")
                    ps = psA.tile([P, T], F32, tag="psA", bufs=4, name="ps_v")
                    psv = ps[:, 0:HPG * D]
                    for e in range(ET):
                        nc.tensor.matmul(
                            psv, h_in[e][:, tt * P:(tt + 1) * P], wv_t[e],
                            start=(e == 0), stop=(e == ET - 1))
                    nc.vector.tensor_add(
                        vt[:, :, 0:D],
                        psv.rearrange("p (h d) -> p h d", d=D),
                        bv_bc[g // 2][:, (g % 2) * HPG * D:((g % 2) + 1) * HPG * D]
                        .rearrange("p (h d) -> p h d", d=D))
                    nc.vector.tensor_copy(vt[:, :, D], ones_sb[:, 0:HPG])
                    v_tiles.append(vt)
                # ---- attention for the 4 heads of this group ----
                for hh in range(HPG):
                    ktile = k_tiles[hh // 2]
                    qtile = q_tiles[hh // 2]
                    row = (hh % 2) * D
                    ps_o = psO.tile([D + 1, T], F32, tag="psO", bufs=2,
                                    name="ps_o")
                    for kt in range(ST):
                        ps_s = psA.tile([P, T], F32, tag="psA", bufs=4,
                                        name="ps_s")
                        nc.tensor.matmul(
                            ps_s, ktile[row:row + D, kt * P:(kt + 1) * P],
                            qtile[row:row + D, :], start=True, stop=True)
                        p_t = pp.tile([P, T], F32R, tag="pp", bufs=2,
                                      name="p_t")
                        nc.vector.tensor_add(p_t, ps_s, masks[kt])
                        nc.scalar.activation(p_t, p_t, AF.Exp)
                        nc.tensor.matmul(ps_o, v_tiles[kt][:, hh, :], p_t,
                                         start=(kt == 0), stop=(kt == ST - 1))
                    rrow = prow.tile([1, T], F32R, tag="row", bufs=4,
                                     name="rrow")
                    with nc.allow_low_precision(reason="softmax recip row"):
                        nc.vector.reciprocal(rrow, ps_o[D:D + 1, :])
                    ps_r = psA.tile([P, T], F32, tag="psA", bufs=4,
                                    name="ps_r")
                    psr = ps_r[0:D, :]
                    nc.tensor.matmul(psr, ones_sb[0:1, 0:D], rrow,
                                     start=True, stop=True)
                    rbc = prbc.tile([D, T], F32, tag="rbc", bufs=1, name="rbc")
                    nc.vector.tensor_copy(rbc, psr)
                    nc.vector.tensor_mul(
                        o_attn[e0 + hh // 2][row:row + D, :],
                        ps_o[0:D, :], rbc)

            # ---- output projection + residual -> res1 ----
            res1 = []
            for fi in range(ET):
                wsl = pw.tile([P, ET, P], F32R, tag="pw", bufs=2,
                              name=f"wo{i}_{fi}")
                nc.sync.dma_start(out=wsl, in_=wo.ap()[i, fi])
                ps = psA.tile([P, T], F32, tag="psA", bufs=4, name="ps_op")
                for e in range(ET):
                    nc.tensor.matmul(ps, wsl[:, e, :], o_attn[e],
                                     start=(e == 0), stop=(e == ET - 1))
                rt = pres.tile([P, T], F32R, tag="pres", bufs=8,
                               name=f"res1_{i}_{fi}")
                nc.vector.scalar_tensor_tensor(
                    out=rt, in0=ps, scalar=b8[:, BO, fi:fi + 1],
                    in1=h_loc[fi], op0=OP.add, op1=OP.add)
                res1.append(rt)

            # ---- LN1 (in place: res1 becomes hln1) ----
            layernorm(res1, b8, L1G, L1B, None)
            hln1 = res1

            # ---- FFN ----
            f_tiles = []
            for j in range(FT):
                wsl = pw.tile([P, ET, P], F32R, tag="pw", bufs=2,
                              name=f"w1_{i}_{j}")
                nc.sync.dma_start(out=wsl, in_=w1.ap()[i, j])
                ps = psA.tile([P, T], F32, tag="psA", bufs=4, name="ps_f1")
                for e in range(ET):
                    nc.tensor.matmul(ps, wsl[:, e, :], hln1[e],
                                     start=(e == 0), stop=(e == ET - 1))
                ft = pf.tile([P, T], BF16, tag="pf", bufs=32, name=f"F{i}_{j}")
                nc.vector.tensor_scalar(
                    out=ft, in0=ps, scalar1=b1t[:, j:j + 1], scalar2=0.0,
                    op0=OP.add, op1=OP.max)
                f_tiles.append(ft)
            res2 = []
            JH = FT // 2
            for e in range(ET):
                ps = psA.tile([P, T], F32, tag="psA", bufs=4, name="ps_f2")
                for jh in range(2):
                    w2sl = pw2.tile([P, JH, P], BF16, tag="pw2", bufs=2,
                                    name=f"w2_{i}_{e}_{jh}")
                    nc.sync.dma_start(
                        out=w2sl,
                        in_=w2.ap()[i, e, jh].rearrange(
                            "p (j m) -> p j m", m=P))
                    for jj in range(JH):
                        j = jh * JH + jj
                        nc.tensor.matmul(ps, w2sl[:, jj, :], f_tiles[j],
                                         start=(j == 0), stop=(j == FT - 1))
                rt = po.tile([P, T], F32R, tag="po", bufs=8,
                             name=f"res2_{i}_{e}")
                nc.vector.scalar_tensor_tensor(
                    out=rt, in0=ps, scalar=b8[:, B2, e:e + 1],
                    in1=hln1[e], op0=OP.add, op1=OP.add)
                res2.append(rt)

            # ---- LN2 -> new local hidden ----
            h_out = [ph_loc.tile([P, T], F32R, tag="hloc", bufs=8,
                                 name=f"hout{i}_{e}") for e in range(ET)]
            layernorm(res2, b8, L2G, L2B, h_out)
            h_loc = h_out

            # ---- AllGather with pair partner (split in 2 for overlap) ----
            EH2 = ET // 2
            for hh2 in range(2):
                for eo in range(EH2):
                    e = hh2 * EH2 + eo
                    nc.sync.dma_start(
                        out=cc_in[i][hh2].ap()[eo * P:(eo + 1) * P, :],
                        in_=h_out[e])
                nc.gpsimd.collective_compute(
                    "AllGather", OP.bypass, replica_groups=groups,
                    ins=[cc_in[i][hh2].ap()], outs=[cc_out[i][hh2].ap()])
            h_in = []
            for e in range(ET):
                hh2, eo = e // EH2, e % EH2
                hf = ph_in.tile([P, S], F32R, tag="hin", bufs=8,
                                name=f"hf{i + 1}_{e}")
                nc.sync.dma_start(
                    out=hf[:, 0:T],
                    in_=cc_out[i][hh2].ap()[0, eo * P:(eo + 1) * P, :])
                nc.sync.dma_start(
                    out=hf[:, T:S],
                    in_=cc_out[i][hh2].ap()[1, eo * P:(eo + 1) * P, :])
                h_in.append(hf)

        # =================== vocab projection ===================
        h_fin = h_in
        for ch in range(NVC):
            wo_t = []
            for e in range(ET):
                wt = pwo.tile([P, VC], F32R, tag="pwo", bufs=8,
                              name=f"wout{ch}_{e}")
                nc.gpsimd.dma_start(out=wt, in_=woutS.ap()[ch, e])
                wo_t.append(wt)
            bo_bc = pres.tile([P, VC], F32, tag="pres", bufs=8,
                              name=f"bout{ch}")
            nc.sync.dma_start(
                out=bo_bc,
                in_=bcast_rows(boutS.ap()[ch * VC:(ch + 1) * VC], P))
            for tt in range(ST):
                ps = psA.tile([P, T], F32, tag="psA", bufs=4, name="ps_vo")
                psl = ps[:, 0:VC]
                for e in range(ET):
                    nc.tensor.matmul(
                        psl, h_fin[e][:, tt * P:(tt + 1) * P], wo_t[e],
                        start=(e == 0), stop=(e == ET - 1))
                ot = po.tile([P, VC], F32, tag="po", bufs=8, name="out_sb")
                nc.vector.tensor_add(ot, psl, bo_bc)
                nc.scalar.dma_start(
                    out=logits.ap()[tt * P:(tt + 1) * P, ch * VC:(ch + 1) * VC],
                    in_=ot)

    nc.finalize()
    return nc


def _posenc():
    even = np.arange(0, E, 2, dtype=np.float32)
    denom = np.power(np.float32(10000.0), even / np.float32(E))
    pos = np.arange(S, dtype=np.float32)[:, None]
    sin = np.sin(pos / denom)
    cos = np.cos(pos / denom)
    return np.stack([sin, cos], axis=2).reshape(S, E).astype(np.float32)


def prep_in_maps(x, emb, Wq, bq, Wk, bk, Wv, bv, Wo, bo,
                 ln1_g, ln1_b, ln2_g, ln2_b, W1, b1, W2, b2, Wout, bout):
    x = np.asarray(x)
    f = lambda a: np.ascontiguousarray(np.asarray(a), dtype=np.float32)
    emb, Wq, bq, Wk, bk = f(emb), f(Wq), f(bq), f(Wk), f(bk)
    Wv, bv, Wo, bo = f(Wv), f(bv), f(Wo), f(bo)
    ln1_g, ln1_b, ln2_g, ln2_b = f(ln1_g), f(ln1_b), f(ln2_g), f(ln2_b)
    W1, b1, W2, b2, Wout, bout = f(W1), f(b1), f(W2), f(b2), f(Wout), f(bout)

    scale = np.float32(1.0) / np.float32(np.sqrt(np.float32(E)))
    h0 = emb[x] + _posenc()[None]              # [NB, S, E] fp32
    wq_s = Wq * scale
    bq_s = bq * scale

    def pack_proj(W):   # [L,E,E] -> [L, fi, p, e, m]
        return np.ascontiguousarray(
            W.reshape(L, ET, P, ET, P).transpose(0, 3, 2, 1, 4))

    wq_t = pack_proj(wq_s)
    wk_t = pack_proj(Wk)
    wo_t = pack_proj(Wo)
    wv_t = np.ascontiguousarray(
        Wv.reshape(L, ET, P, NG, HPG * D).transpose(0, 3, 1, 2, 4))
    w1_t = np.ascontiguousarray(
        W1.reshape(L, ET, P, FT, P).transpose(0, 3, 2, 1, 4))
    JH = FT // 2
    w2_t = np.ascontiguousarray(
        W2.reshape(L, 2, JH, P, ET, P)          # [l, jh, jj, p, e, m]
        .transpose(0, 4, 1, 3, 2, 5)            # [l, e, jh, p, jj, m]
        .reshape(L, ET, 2, P, JH * P)
        .astype(ml_dtypes.bfloat16))
    bias8 = np.ascontiguousarray(
        np.stack([bq_s, bk, bo, b2, ln1_g, ln1_b, ln2_g, ln2_b], axis=1)
        .astype(np.float32).reshape(L, 8, ET, P).transpose(0, 3, 1, 2))
    b1_t = np.ascontiguousarray(
        b1.reshape(L, FT, P).transpose(0, 2, 1).astype(np.float32))
    onesc = np.ones((P, P), np.float32)

    kidx = np.arange(S, dtype=np.int64)[:, None]
    qbase = np.arange(T, dtype=np.int64)[None, :]
    mask_half = []
    for half in range(2):
        m = np.where(kidx <= qbase + half * T, 0.0, MASKVAL).astype(np.float32)
        mask_half.append(m.astype(ml_dtypes.bfloat16))   # [S, T]

    in_maps = []
    for c in range(NCORES):
        b, half = c // 2, c % 2
        h0T = np.ascontiguousarray(h0[b].T)              # [E, S]
        in_maps.append({
            "h0loc": np.ascontiguousarray(h0T[:, half * T:(half + 1) * T]),
            "h0full": h0T,
            "maskT": mask_half[half],
            "wq": wq_t, "wk": wk_t, "wv": wv_t, "wo": wo_t,
            "w1": w1_t, "w2": w2_t,
            "bias8": bias8, "b1v": b1_t, "bvv": bv,
            "woutS": np.ascontiguousarray(
                Wout[:, half * VS:(half + 1) * VS]
                .reshape(ET, P, NVC, VC).transpose(2, 0, 1, 3)),
            "boutS": np.ascontiguousarray(bout[half * VS:(half + 1) * VS]),
            "onesc": onesc,
        })

    return in_maps


def assemble(results):
    out = np.empty((NB, S, V), np.float32)
    for c in range(NCORES):
        b, half = c // 2, c % 2
        out[b, :, half * VS:(half + 1) * VS] = results[c]["logits"]
    return out


def get_nc():
    if "nc" not in _CACHE:
        _CACHE["nc"] = _build()
    return _CACHE["nc"]


def kernel(**inputs):
    global LAST_RESULTS
    nc = get_nc()
    in_maps = prep_in_maps(**inputs)
    res = run_bass_kernel_spmd(nc, in_maps, core_ids=list(range(NCORES)))
    LAST_RESULTS = res
    return assemble(res.results)
